# revision 42
# baseline (speedup 1.0000x reference)
"""Trainium2 Bass kernel for nn_Encoder (dense MLP with stochastic ternarization).

y = tanh(x @ (s1*T(w1,n1)) + b1) @ (s2*T(w2,n2)) + b2,  T(w,n) = (w-n>1) - (w-n<-1)

Sharding: tensor-parallel over the 16384 hidden dim across 8 cores. Each core
gets a 2048-wide hidden shard of w1/noise1/s1/b1 (column-sharded) and the
matching 2048-row shard of w2/noise2; x is replicated (host pre-transposed to
bf16, tiled per 512-batch block).

Kernel structure:
- Ternary weights in fp8e4 ({-2,0,+2} exact); PE takes fp8 stationary x bf16
  moving. 1 MiB contiguous weight DMA blocks (host pre-tiled); the first
  supply quarter is halved so the first matmul chains unblock early.
- Blocks 0/1 interleave layer-1 m-groups at supply-stripe granularity so PE
  consumption tracks the ternarize supply; blocks 2/3 run from resident
  weights.
- h stays in SBUF between layers. s2/b2 are applied at PSUM evacuation (b2 on
  core 0 only) and partials are stored bf16; each (block, half) goes through
  its own bf16 ReduceScatter(add) into DRAM scratch, DMA-copied to the output
  from the gpsimd queue (a sync-queue copy would head-of-line block y stores).
  Block 3 splits its second half into two quarter-RS so the serial tail after
  the last matmul is short, and two 1 KiB pre-sync collectives during L1(b3)
  absorb accumulated inter-core skew before the final RS burst.

Ternarization: q = w - noise (DVE), tanh(2^30*(q-1)) + tanh(2^30*(q+1)) (ACT)
== (q>1)-(q<-1) doubled; the factor 2 is folded into s1/s2 on the host.
"""

import sys

for _p in ("/opt/trn_rl_repo",):
    if _p not in sys.path:
        sys.path.insert(0, _p)

import numpy as np
import ml_dtypes

import concourse.bass as bass
import concourse.bacc as bacc
import concourse.mybir as mybir
import concourse.tile as tile
import concourse.bass_utils as _bass_utils
from concourse.bass_utils import run_bass_kernel_spmd

BF16 = mybir.dt.bfloat16
F32 = mybir.dt.float32
FP8 = mybir.dt.float8e4
NPBF16 = ml_dtypes.bfloat16

N_CORES = 8
B = 2048
DIN = 3072
DHID = 16384
DOUT = 1024
HSH = DHID // N_CORES   # 2048
DSH = DOUT // N_CORES   # 128

K1 = DIN // 128          # 24 contraction tiles, layer 1
KG1 = K1 // 4            # 6 groups of 4 k-tiles (1 MiB weight blocks)
K2 = HSH // 128          # 16 contraction tiles, layer 2
KG2 = K2 // 2            # 8 groups of 2 k2-tiles (1 MiB weight blocks)
NB = B // 512            # 4 batch blocks
MT = HSH // 128          # 16 hidden m-tiles
ND = DOUT // 128         # 8 dout tiles
QW = 512                 # ternarize quarter width
NQ = HSH // QW           # 4 quarters

BIGK = float(2 ** 30)

TANH = mybir.ActivationFunctionType.Tanh
MULT = mybir.AluOpType.mult
ADD = mybir.AluOpType.add


def build_bass():
    nc = bacc.Bacc("TRN2", target_bir_lowering=False, debug=False, num_devices=N_CORES)

    xtb = nc.dram_tensor("xtb", [NB, 128, K1, 512], BF16, kind="ExternalInput")
    # cols 0:512 as two contiguous 256-wide stripes (the first quarter halved
    # for an early start, but host-packed contiguous for full DMA bandwidth),
    # then full 512-wide quarters
    w1a = nc.dram_tensor("w1a", [2, KG1, 128, 4, 256], F32, kind="ExternalInput")
    n1a = nc.dram_tensor("n1a", [2, KG1, 128, 4, 256], F32, kind="ExternalInput")
    w1r = nc.dram_tensor("w1r", [3, KG1, 128, 4, QW], F32, kind="ExternalInput")
    n1r = nc.dram_tensor("n1r", [3, KG1, 128, 4, QW], F32, kind="ExternalInput")
    s1h = nc.dram_tensor("s1h", [128, MT], F32, kind="ExternalInput")
    b1m = nc.dram_tensor("b1m", [128, MT], F32, kind="ExternalInput")
    w2g = nc.dram_tensor("w2g", [KG2, 128, 2, DOUT], F32, kind="ExternalInput")
    n2g = nc.dram_tensor("n2g", [KG2, 128, 2, DOUT], F32, kind="ExternalInput")
    s2d = nc.dram_tensor("s2d", [128, ND], F32, kind="ExternalInput")
    b2d = nc.dram_tensor("b2d", [128, ND], F32, kind="ExternalInput")

    # per-core output rows for block b:
    #   rows  0:64   = chunk of douts    0:512  (RS over d-tiles 0-3)
    #   rows 64:128  = chunk of douts  512:1024
    #     blocks 0-2: one 512-row RS (chunk 64)
    #     block 3: two 256-row RS (chunks 32+32) for a shorter serial tail
    yo = nc.dram_tensor("yo", [NB, 128, 512], BF16, kind="ExternalOutput")

    with tile.TileContext(nc) as tc:
        with (
            tc.tile_pool(name="const", bufs=1) as cpool,
            tc.tile_pool(name="dram", bufs=1, space="DRAM") as dpool,
            tc.tile_pool(name="t2w1", bufs=KG1) as t2pool,
            tc.tile_pool(name="t2w2", bufs=1) as t22pool,
            tc.tile_pool(name="stage", bufs=2) as spool,
            tc.tile_pool(name="xtn", bufs=2) as xpool,
            tc.tile_pool(name="hblk", bufs=2 * MT) as hpool,
            tc.tile_pool(name="yblk", bufs=4) as ypool,
            tc.tile_pool(name="ps1", bufs=6, space="PSUM") as pspool,
            tc.tile_pool(name="ps2", bufs=2, space="PSUM") as ps2pool,
        ):
            s1_sb = cpool.tile([128, MT], F32, tag="s1")
            b1_sb = cpool.tile([128, MT], F32, tag="b1")
            s2_sb = cpool.tile([128, ND], F32, tag="s2")
            b2_sb = cpool.tile([128, ND], F32, tag="b2")
            nc.scalar.dma_start(s1_sb[:], s1h[:, :])
            nc.scalar.dma_start(b1_sb[:], b1m[:, :])
            nc.scalar.dma_start(s2_sb[:], s2d[:, :])
            nc.scalar.dma_start(b2_sb[:], b2d[:, :])
            kneg = cpool.tile([128, 1], F32, tag="kneg")
            nc.vector.memset(kneg[:], -BIGK)
            kpos = cpool.tile([128, 1], F32, tag="kpos")
            nc.vector.memset(kpos[:], BIGK)

            # RS inputs/outputs: (block, segment). Block 3 reduces in ONE op:
            # per-op latency dominates at these sizes, so one 1 MiB RS beats
            # a serialized chain of smaller ones in the tail.
            def seg_rows(b):
                return [512, 512] if b < 3 else [1024]

            yq = [[dpool.tile([r, 512], BF16, tag=f"yq{b}{i}",
                              name=f"yq_b{b}s{i}")
                   for i, r in enumerate(seg_rows(b))] for b in range(NB)]
            ro = [[dpool.tile([r // N_CORES, 512], BF16, tag=f"ro{b}{i}",
                              name=f"ro_b{b}s{i}")
                   for i, r in enumerate(seg_rows(b))] for b in range(NB)]
            dsrc = [dpool.tile([8, 64], BF16, tag=f"dsrc{i}",
                               name=f"dsrc{i}") for i in range(2)]
            ddst = [dpool.tile([1, 64], BF16, tag=f"ddst{i}",
                               name=f"ddst{i}") for i in range(2)]

            xtn_tiles = {}
            for b in (0, 1):
                xtn_tiles[b] = xpool.tile([128, K1, 512], BF16, tag="xtn",
                                          name=f"xtn{b}")
                nc.scalar.dma_start(xtn_tiles[b][:], xtb[b])

            # ---- ternarize (1 MiB blocks) ----
            t2g = [t2pool.tile([128, 4, HSH], FP8, tag="t2", name=f"t2g_{kg}")
                   for kg in range(KG1)]
            t22 = t22pool.tile([128, K2, DOUT], FP8, tag="t22")

            def tern_block(dst_ap, w_src, n_src, sub_k, fd):
                w_t = spool.tile([128, sub_k, fd], F32, tag="w")
                nc.sync.dma_start(w_t[:], w_src)
                n_t = spool.tile([128, sub_k, fd], F32, tag="n")
                nc.sync.dma_start(n_t[:], n_src)
                nc.vector.tensor_sub(w_t[:], w_t[:], n_t[:])
                a1 = spool.tile([128, sub_k, fd], FP8, tag="a1")
                nc.scalar.activation(a1[:], w_t[:], TANH, bias=kneg[:, 0:1], scale=BIGK)
                a2 = spool.tile([128, sub_k, fd], FP8, tag="a2")
                nc.scalar.activation(a2[:], w_t[:], TANH, bias=kpos[:, 0:1], scale=BIGK)
                nc.vector.tensor_add(dst_ap, a1[:], a2[:])

            def presync(i, dep_tile):
                # tiny store creates the trigger-time dependency; the 1 KiB
                # ReduceScatter syncs the 8 cores' CC streams under compute
                nc.sync.dma_start(dsrc[i][:], dep_tile[0:8, 0:64])
                nc.gpsimd.collective_compute(
                    "ReduceScatter",
                    mybir.AluOpType.add,
                    replica_groups=[list(range(N_CORES))],
                    ins=[dsrc[i].opt()],
                    outs=[ddst[i].opt()],
                )

            # supply stripes: 256-wide contiguous blocks first (fast start),
            # then full 512-wide quarters
            SUPPLY = [("a", 0, 0, 256), ("a", 1, 256, 256),
                      ("r", 0, 512, 512), ("r", 1, 1024, 512), ("r", 2, 1536, 512)]
            srcs = {"a": (w1a, n1a), "r": (w1r, n1r)}
            for t, i, c0, cw in SUPPLY:
                ws, ns = srcs[t]
                for kg in range(KG1):
                    tern_block(
                        t2g[kg][:, :, c0:c0 + cw],
                        ws[i, kg], ns[i, kg], 4, cw,
                    )
            for kg in range(KG2):
                tern_block(
                    t22[:, kg * 2:(kg + 1) * 2, :],
                    w2g[kg], n2g[kg], 2, DOUT,
                )

            # ---- compute ----
            h_sets = {0: [], 1: [], 2: [], 3: []}

            MGROUPS = [(0, 2), (2, 2), (4, 4), (8, 4), (12, 4)]

            def layer1_mgroup(b, g):
                xtn = xtn_tiles[b]
                m0, mw = MGROUPS[g]
                for m in range(m0, m0 + mw):
                    ps = pspool.tile([128, 512], F32, tag="ps")
                    for k in range(K1):
                        nc.tensor.matmul(
                            ps[:],
                            t2g[k // 4][:, k % 4, m * 128:(m + 1) * 128],
                            xtn[:, k, :],
                            start=(k == 0), stop=(k == K1 - 1))
                    h_m = hpool.tile([128, 512], BF16, tag="h")
                    nc.scalar.activation(
                        h_m[:], ps[:], TANH,
                        bias=b1_sb[:, m:m + 1], scale=s1_sb[:, m:m + 1],
                    )
                    h_sets[b].append(h_m)

            def layer2_block(b):
                if b < 3:
                    seg_of_d = [0, 0, 0, 0, 1, 1, 1, 1]
                    part_of_d = [0, 1, 2, 3, 0, 1, 2, 3]
                    last_of_seg = [3, 7]
                else:
                    seg_of_d = [0] * 8
                    part_of_d = list(range(8))
                    last_of_seg = [7]
                for d in range(ND):
                    p = ps2pool.tile([128, 512], F32, tag="ps2")
                    for k2 in range(K2):
                        nc.tensor.matmul(p[:], t22[:, k2, d * 128:(d + 1) * 128],
                                         h_sets[b][k2][:],
                                         start=(k2 == 0), stop=(k2 == K2 - 1))
                    y_sb = ypool.tile([128, 512], BF16, tag="y")
                    nc.vector.tensor_scalar(
                        y_sb[:], p[:], s2_sb[:, d:d + 1], b2_sb[:, d:d + 1],
                        MULT, ADD,
                    )
                    seg, part = seg_of_d[d], part_of_d[d]
                    nc.scalar.dma_start(
                        yq[b][seg][part * 128:(part + 1) * 128, :], y_sb[:],
                    )
                    if d in last_of_seg:
                        nc.gpsimd.collective_compute(
                            "ReduceScatter",
                            mybir.AluOpType.add,
                            replica_groups=[list(range(N_CORES))],
                            ins=[yq[b][seg].opt()],
                            outs=[ro[b][seg].opt()],
                        )
                # yo copies last on the gpsimd queue (they wait on RS
                # completion; nothing time-critical queues behind them there)
                r0 = 0
                for i, r in enumerate(seg_rows(b)):
                    rc = r // N_CORES
                    nc.gpsimd.dma_start(yo[b, r0:r0 + rc, :], ro[b][i][:])
                    r0 += rc

            # blocks 0/1 with supply-interleaved layer-1
            for g in range(len(MGROUPS)):
                layer1_mgroup(0, g)
                layer1_mgroup(1, g)
            layer2_block(0)
            layer2_block(1)

            # blocks 2/3 from resident weights
            for b in (2, 3):
                xtn_tiles[b] = xpool.tile([128, K1, 512], BF16, tag="xtn",
                                          name=f"xtn{b}")
                nc.scalar.dma_start(xtn_tiles[b][:], xtb[b])
                for g in range(len(MGROUPS)):
                    layer1_mgroup(b, g)
                    if b == 3 and g in (0, 4):
                        presync(0 if g == 0 else 1, h_sets[3][-1])
                layer2_block(b)
            # g==4 ends at m11; the second presync completes under the m12-15
            # and L2(b3) compute, so the final RS pays only residual skew

    nc.compile()
    return nc


_NC_CACHE = {}


def _get_nc():
    if "nc" not in _NC_CACHE:
        _NC_CACHE["nc"] = build_bass()
    return _NC_CACHE["nc"]


def _make_in_maps(x, w1, s1, b1, w2, s2, b2, noise1, noise2):
    x = np.asarray(x, dtype=np.float32)
    w1 = np.asarray(w1, dtype=np.float32)
    s1 = np.asarray(s1, dtype=np.float32)
    b1 = np.asarray(b1, dtype=np.float32)
    w2 = np.asarray(w2, dtype=np.float32)
    s2 = np.asarray(s2, dtype=np.float32)
    b2 = np.asarray(b2, dtype=np.float32)
    noise1 = np.asarray(noise1, dtype=np.float32)
    noise2 = np.asarray(noise2, dtype=np.float32)

    xT = x.T.astype(NPBF16)
    xtb = np.ascontiguousarray(xT.reshape(K1, 128, NB, 512).transpose(2, 1, 0, 3))

    def w1_tile(w):   # [din, HSH] -> (a [2,KG1,128,4,256], r [3,KG1,128,4,512])
        wk = w.reshape(KG1, 4, 128, HSH)
        a = np.empty((2, KG1, 128, 4, 256), dtype=np.float32)
        r = np.empty((3, KG1, 128, 4, QW), dtype=np.float32)
        for i in range(2):
            a[i] = wk[:, :, :, i * 256:(i + 1) * 256].transpose(0, 2, 1, 3)
        for i in range(3):
            r[i] = wk[:, :, :, 512 + i * 512:512 + (i + 1) * 512].transpose(0, 2, 1, 3)
        return np.ascontiguousarray(a), np.ascontiguousarray(r)

    def w2_tile(w):   # [HSH, DOUT] -> [KG2, 128, 2, DOUT]
        return np.ascontiguousarray(
            w.reshape(KG2, 2, 128, DOUT).transpose(0, 2, 1, 3))

    in_maps = []
    for c in range(N_CORES):
        hs = slice(c * HSH, (c + 1) * HSH)
        s2m = np.ascontiguousarray((0.5 * s2).reshape(ND, 128).T)
        b2m = np.ascontiguousarray(b2.reshape(ND, 128).T) if c == 0 else \
            np.zeros((128, ND), dtype=np.float32)
        w1a, w1r = w1_tile(np.ascontiguousarray(w1[:, hs]))
        n1a, n1r = w1_tile(np.ascontiguousarray(noise1[:, hs]))
        in_maps.append({
            "xtb": xtb,
            "w1a": w1a, "w1r": w1r,
            "n1a": n1a, "n1r": n1r,
            "s1h": np.ascontiguousarray((0.5 * s1[hs]).reshape(MT, 128).T),
            "b1m": np.ascontiguousarray(b1[hs].reshape(MT, 128).T),
            "w2g": w2_tile(np.ascontiguousarray(w2[hs, :])),
            "n2g": w2_tile(np.ascontiguousarray(noise2[hs, :])),
            "s2d": s2m,
            "b2d": b2m,
        })
    return in_maps


def kernel(x, w1, s1, b1, w2, s2, b2, noise1, noise2, _bench_out=None):
    """Full-input, full-output entry point. Shards across 8 NeuronCores."""
    nc = _get_nc()
    in_maps = _make_in_maps(x, w1, s1, b1, w2, s2, b2, noise1, noise2)
    res = run_bass_kernel_spmd(nc, in_maps, core_ids=list(range(N_CORES)))
    if _bench_out is not None:
        _bench_out.append(res)
    yT = np.empty((DOUT, B), dtype=np.float32)
    for c in range(N_CORES):
        out_c = np.asarray(res.results[c]["yo"]).astype(np.float32)
        for b in range(NB):
            cols = slice(b * 512, (b + 1) * 512)
            if b < 3:
                yT[c * 64:(c + 1) * 64, cols] = out_c[b, 0:64]
                yT[512 + c * 64:512 + (c + 1) * 64, cols] = out_c[b, 64:128]
            else:
                yT[c * 128:(c + 1) * 128, cols] = out_c[b]
    return np.ascontiguousarray(yT.T).astype(np.float32)


if __name__ == "__main__":
    nc = build_bass()
    print("built OK")


# revision 49
# speedup vs baseline: 1.0059x; 1.0059x over previous
"""Trainium2 Bass kernel for nn_Encoder (dense MLP with stochastic ternarization).

y = tanh(x @ (s1*T(w1,n1)) + b1) @ (s2*T(w2,n2)) + b2,  T(w,n) = (w-n>1) - (w-n<-1)

Sharding: tensor-parallel over the 16384 hidden dim across 8 cores. Each core
gets a 2048-wide hidden shard of w1/noise1/s1/b1 (column-sharded) and the
matching 2048-row shard of w2/noise2; x is replicated (host pre-transposed to
bf16, tiled per 512-batch block).

Kernel structure:
- Ternary weights in fp8e4 ({-2,0,+2} exact); PE takes fp8 stationary x bf16
  moving. 1 MiB contiguous weight DMA blocks (host pre-tiled); the first
  supply quarter is halved so the first matmul chains unblock early.
- Blocks 0/1 interleave layer-1 m-groups at supply-stripe granularity so PE
  consumption tracks the ternarize supply; blocks 2/3 run from resident
  weights.
- h stays in SBUF between layers. s2/b2 are applied at PSUM evacuation (b2 on
  core 0 only) and partials are stored bf16; each (block, half) goes through
  its own bf16 ReduceScatter(add) into DRAM scratch, DMA-copied to the output
  from the gpsimd queue (a sync-queue copy would head-of-line block y stores).
  Block 3 reduces in a single RS (per-op latency dominates at these sizes, so
  one op beats a serialized chain in the tail), and two 1 KiB pre-sync
  collectives during L1(b3) absorb accumulated inter-core skew beforehand.

Ternarization: q = w - noise (DVE), tanh(2^30*(q-1)) + tanh(2^30*(q+1)) (ACT)
== (q>1)-(q<-1) doubled; the factor 2 is folded into s1/s2 on the host.
"""

import sys

for _p in ("/opt/trn_rl_repo",):
    if _p not in sys.path:
        sys.path.insert(0, _p)

import numpy as np
import ml_dtypes

import concourse.bass as bass
import concourse.bacc as bacc
import concourse.mybir as mybir
import concourse.tile as tile
import concourse.bass_utils as _bass_utils
from concourse.bass_utils import run_bass_kernel_spmd

BF16 = mybir.dt.bfloat16
F32 = mybir.dt.float32
FP8 = mybir.dt.float8e4
NPBF16 = ml_dtypes.bfloat16

N_CORES = 8
B = 2048
DIN = 3072
DHID = 16384
DOUT = 1024
HSH = DHID // N_CORES   # 2048
DSH = DOUT // N_CORES   # 128

K1 = DIN // 128          # 24 contraction tiles, layer 1
KG1 = K1 // 4            # 6 groups of 4 k-tiles (1 MiB weight blocks)
K2 = HSH // 128          # 16 contraction tiles, layer 2
KG2 = K2 // 2            # 8 groups of 2 k2-tiles (1 MiB weight blocks)
NB = B // 512            # 4 batch blocks
MT = HSH // 128          # 16 hidden m-tiles
ND = DOUT // 128         # 8 dout tiles
QW = 512                 # ternarize quarter width
NQ = HSH // QW           # 4 quarters

BIGK = float(2 ** 30)

TANH = mybir.ActivationFunctionType.Tanh
MULT = mybir.AluOpType.mult
ADD = mybir.AluOpType.add


def build_bass():
    nc = bacc.Bacc("TRN2", target_bir_lowering=False, debug=False, num_devices=N_CORES)

    xtb = nc.dram_tensor("xtb", [NB, 128, K1, 512], BF16, kind="ExternalInput")
    # cols 0:256 as two contiguous 128-wide stripes, 256:512 as one 256-wide,
    # then full 512-wide quarters: contiguous DMA blocks in supply order
    w1a = nc.dram_tensor("w1a", [2, KG1, 128, 4, 128], F32, kind="ExternalInput")
    n1a = nc.dram_tensor("n1a", [2, KG1, 128, 4, 128], F32, kind="ExternalInput")
    w1b = nc.dram_tensor("w1b", [1, KG1, 128, 4, 256], F32, kind="ExternalInput")
    n1b = nc.dram_tensor("n1b", [1, KG1, 128, 4, 256], F32, kind="ExternalInput")
    w1r = nc.dram_tensor("w1r", [3, KG1, 128, 4, QW], F32, kind="ExternalInput")
    n1r = nc.dram_tensor("n1r", [3, KG1, 128, 4, QW], F32, kind="ExternalInput")
    s1h = nc.dram_tensor("s1h", [128, MT], F32, kind="ExternalInput")
    b1m = nc.dram_tensor("b1m", [128, MT], F32, kind="ExternalInput")
    w2g = nc.dram_tensor("w2g", [KG2, 128, 2, DOUT], F32, kind="ExternalInput")
    n2g = nc.dram_tensor("n2g", [KG2, 128, 2, DOUT], F32, kind="ExternalInput")
    s2d = nc.dram_tensor("s2d", [128, ND], F32, kind="ExternalInput")
    b2d = nc.dram_tensor("b2d", [128, ND], F32, kind="ExternalInput")

    # per-core output rows for block b:
    #   blocks 0-2: rows 0:64 = chunk of douts 0:512 (RS over d-tiles 0-3),
    #               rows 64:128 = chunk of douts 512:1024
    #   block 3: rows 0:128 = chunk of douts 0:1024 (single RS)
    yo = nc.dram_tensor("yo", [NB, 128, 512], BF16, kind="ExternalOutput")

    with tile.TileContext(nc) as tc:
        with (
            tc.tile_pool(name="const", bufs=1) as cpool,
            tc.tile_pool(name="dram", bufs=1, space="DRAM") as dpool,
            tc.tile_pool(name="t2w1", bufs=KG1) as t2pool,
            tc.tile_pool(name="t2w2", bufs=1) as t22pool,
            tc.tile_pool(name="stage", bufs=2) as spool,
            tc.tile_pool(name="xtn", bufs=2) as xpool,
            tc.tile_pool(name="hblk", bufs=2 * MT) as hpool,
            tc.tile_pool(name="yblk", bufs=4) as ypool,
            tc.tile_pool(name="ps1", bufs=6, space="PSUM") as pspool,
            tc.tile_pool(name="ps2", bufs=2, space="PSUM") as ps2pool,
        ):
            s1_sb = cpool.tile([128, MT], F32, tag="s1")
            b1_sb = cpool.tile([128, MT], F32, tag="b1")
            s2_sb = cpool.tile([128, ND], F32, tag="s2")
            b2_sb = cpool.tile([128, ND], F32, tag="b2")
            nc.scalar.dma_start(s1_sb[:], s1h[:, :])
            nc.scalar.dma_start(b1_sb[:], b1m[:, :])
            nc.scalar.dma_start(s2_sb[:], s2d[:, :])
            nc.scalar.dma_start(b2_sb[:], b2d[:, :])
            kneg = cpool.tile([128, 1], F32, tag="kneg")
            nc.vector.memset(kneg[:], -BIGK)
            kpos = cpool.tile([128, 1], F32, tag="kpos")
            nc.vector.memset(kpos[:], BIGK)

            # RS inputs/outputs: (block, segment). Two halves per block: the
            # first half's RS overlaps the second half's d-tile compute, so
            # the serial tail is one 512 KiB op instead of a 1 MiB op.
            def seg_rows(b):
                return [512, 512]

            yq = [[dpool.tile([r, 512], BF16, tag=f"yq{b}{i}",
                              name=f"yq_b{b}s{i}")
                   for i, r in enumerate(seg_rows(b))] for b in range(NB)]
            ro = [[dpool.tile([r // N_CORES, 512], BF16, tag=f"ro{b}{i}",
                              name=f"ro_b{b}s{i}")
                   for i, r in enumerate(seg_rows(b))] for b in range(NB)]
            dsrc = [dpool.tile([8, 64], BF16, tag=f"dsrc{i}",
                               name=f"dsrc{i}") for i in range(2)]
            ddst = [dpool.tile([1, 64], BF16, tag=f"ddst{i}",
                               name=f"ddst{i}") for i in range(2)]

            xtn_tiles = {}
            for b in (0, 1):
                xtn_tiles[b] = xpool.tile([128, K1, 512], BF16, tag="xtn",
                                          name=f"xtn{b}")
                nc.scalar.dma_start(xtn_tiles[b][:], xtb[b])

            # ---- ternarize (1 MiB blocks) ----
            t2g = [t2pool.tile([128, 4, HSH], FP8, tag="t2", name=f"t2g_{kg}")
                   for kg in range(KG1)]
            t22 = t22pool.tile([128, K2, DOUT], FP8, tag="t22")

            def tern_block(dst_ap, w_src, n_src, sub_k, fd):
                w_t = spool.tile([128, sub_k, fd], F32, tag="w")
                nc.sync.dma_start(w_t[:], w_src)
                n_t = spool.tile([128, sub_k, fd], F32, tag="n")
                nc.sync.dma_start(n_t[:], n_src)
                nc.vector.tensor_sub(w_t[:], w_t[:], n_t[:])
                a1 = spool.tile([128, sub_k, fd], FP8, tag="a1")
                nc.scalar.activation(a1[:], w_t[:], TANH, bias=kneg[:, 0:1], scale=BIGK)
                a2 = spool.tile([128, sub_k, fd], FP8, tag="a2")
                nc.scalar.activation(a2[:], w_t[:], TANH, bias=kpos[:, 0:1], scale=BIGK)
                nc.vector.tensor_add(dst_ap, a1[:], a2[:])

            def presync(i, dep_tile):
                # tiny store creates the trigger-time dependency; the 1 KiB
                # ReduceScatter syncs the 8 cores' CC streams under compute
                nc.sync.dma_start(dsrc[i][:], dep_tile[0:8, 0:64])
                nc.gpsimd.collective_compute(
                    "ReduceScatter",
                    mybir.AluOpType.add,
                    replica_groups=[list(range(N_CORES))],
                    ins=[dsrc[i].opt()],
                    outs=[ddst[i].opt()],
                )

            # supply stripes: narrow contiguous blocks first (fast start),
            # then full 512-wide quarters
            SUPPLY = [("a", 0, 0, 128), ("a", 1, 128, 128), ("b", 0, 256, 256),
                      ("r", 0, 512, 512), ("r", 1, 1024, 512), ("r", 2, 1536, 512)]
            srcs = {"a": (w1a, n1a), "b": (w1b, n1b), "r": (w1r, n1r)}
            for t, i, c0, cw in SUPPLY:
                ws, ns = srcs[t]
                for kg in range(KG1):
                    tern_block(
                        t2g[kg][:, :, c0:c0 + cw],
                        ws[i, kg], ns[i, kg], 4, cw,
                    )
            for kg in range(KG2):
                tern_block(
                    t22[:, kg * 2:(kg + 1) * 2, :],
                    w2g[kg], n2g[kg], 2, DOUT,
                )

            # ---- compute ----
            h_sets = {0: [], 1: [], 2: [], 3: []}

            MGROUPS = [(0, 1), (1, 1), (2, 2), (4, 4), (8, 4), (12, 4)]

            def layer1_mgroup(b, g):
                xtn = xtn_tiles[b]
                m0, mw = MGROUPS[g]
                for m in range(m0, m0 + mw):
                    ps = pspool.tile([128, 512], F32, tag="ps")
                    for k in range(K1):
                        nc.tensor.matmul(
                            ps[:],
                            t2g[k // 4][:, k % 4, m * 128:(m + 1) * 128],
                            xtn[:, k, :],
                            start=(k == 0), stop=(k == K1 - 1))
                    h_m = hpool.tile([128, 512], BF16, tag="h")
                    nc.scalar.activation(
                        h_m[:], ps[:], TANH,
                        bias=b1_sb[:, m:m + 1], scale=s1_sb[:, m:m + 1],
                    )
                    h_sets[b].append(h_m)

            def layer2_block(b):
                seg_of_d = [0, 0, 0, 0, 1, 1, 1, 1]
                part_of_d = [0, 1, 2, 3, 0, 1, 2, 3]
                last_of_seg = [3, 7]
                for d in range(ND):
                    p = ps2pool.tile([128, 512], F32, tag="ps2")
                    for k2 in range(K2):
                        nc.tensor.matmul(p[:], t22[:, k2, d * 128:(d + 1) * 128],
                                         h_sets[b][k2][:],
                                         start=(k2 == 0), stop=(k2 == K2 - 1))
                    y_sb = ypool.tile([128, 512], BF16, tag="y")
                    nc.vector.tensor_scalar(
                        y_sb[:], p[:], s2_sb[:, d:d + 1], b2_sb[:, d:d + 1],
                        MULT, ADD,
                    )
                    seg, part = seg_of_d[d], part_of_d[d]
                    nc.scalar.dma_start(
                        yq[b][seg][part * 128:(part + 1) * 128, :], y_sb[:],
                    )
                    if d in last_of_seg:
                        nc.gpsimd.collective_compute(
                            "ReduceScatter",
                            mybir.AluOpType.add,
                            replica_groups=[list(range(N_CORES))],
                            ins=[yq[b][seg].opt()],
                            outs=[ro[b][seg].opt()],
                        )
                # yo copies last on the gpsimd queue (they wait on RS
                # completion; nothing time-critical queues behind them there)
                r0 = 0
                for i, r in enumerate(seg_rows(b)):
                    rc = r // N_CORES
                    nc.gpsimd.dma_start(yo[b, r0:r0 + rc, :], ro[b][i][:])
                    r0 += rc

            # blocks 0/1 with supply-interleaved layer-1
            for g in range(len(MGROUPS)):
                layer1_mgroup(0, g)
                layer1_mgroup(1, g)
            layer2_block(0)
            layer2_block(1)

            # blocks 2/3 from resident weights
            for b in (2, 3):
                xtn_tiles[b] = xpool.tile([128, K1, 512], BF16, tag="xtn",
                                          name=f"xtn{b}")
                nc.scalar.dma_start(xtn_tiles[b][:], xtb[b])
                for g in range(len(MGROUPS)):
                    layer1_mgroup(b, g)
                    # presync early enough that its CC op completes before
                    # the block's first RS data is ready (an in-flight CC op
                    # delays the next trigger — seen when presync ran at g4)
                    if b == 3 and g in (0, 3):
                        presync(0 if g == 0 else 1, h_sets[3][-1])
                layer2_block(b)
            # g==4 ends at m11; the second presync completes under the m12-15
            # and L2(b3) compute, so the final RS pays only residual skew

    nc.compile()
    return nc


_NC_CACHE = {}


def _get_nc():
    if "nc" not in _NC_CACHE:
        _NC_CACHE["nc"] = build_bass()
    return _NC_CACHE["nc"]


def _make_in_maps(x, w1, s1, b1, w2, s2, b2, noise1, noise2):
    x = np.asarray(x, dtype=np.float32)
    w1 = np.asarray(w1, dtype=np.float32)
    s1 = np.asarray(s1, dtype=np.float32)
    b1 = np.asarray(b1, dtype=np.float32)
    w2 = np.asarray(w2, dtype=np.float32)
    s2 = np.asarray(s2, dtype=np.float32)
    b2 = np.asarray(b2, dtype=np.float32)
    noise1 = np.asarray(noise1, dtype=np.float32)
    noise2 = np.asarray(noise2, dtype=np.float32)

    xT = x.T.astype(NPBF16)
    xtb = np.ascontiguousarray(xT.reshape(K1, 128, NB, 512).transpose(2, 1, 0, 3))

    def w1_tile(w):   # [din, HSH] -> (a [2,KG1,128,4,128], b [1,...,256], r [3,...,512])
        wk = w.reshape(KG1, 4, 128, HSH)
        a = np.empty((2, KG1, 128, 4, 128), dtype=np.float32)
        bt = np.empty((1, KG1, 128, 4, 256), dtype=np.float32)
        r = np.empty((3, KG1, 128, 4, QW), dtype=np.float32)
        for i in range(2):
            a[i] = wk[:, :, :, i * 128:(i + 1) * 128].transpose(0, 2, 1, 3)
        bt[0] = wk[:, :, :, 256:512].transpose(0, 2, 1, 3)
        for i in range(3):
            r[i] = wk[:, :, :, 512 + i * 512:512 + (i + 1) * 512].transpose(0, 2, 1, 3)
        return (np.ascontiguousarray(a), np.ascontiguousarray(bt),
                np.ascontiguousarray(r))

    def w2_tile(w):   # [HSH, DOUT] -> [KG2, 128, 2, DOUT]
        return np.ascontiguousarray(
            w.reshape(KG2, 2, 128, DOUT).transpose(0, 2, 1, 3))

    in_maps = []
    for c in range(N_CORES):
        hs = slice(c * HSH, (c + 1) * HSH)
        s2m = np.ascontiguousarray((0.5 * s2).reshape(ND, 128).T)
        b2m = np.ascontiguousarray(b2.reshape(ND, 128).T) if c == 0 else \
            np.zeros((128, ND), dtype=np.float32)
        w1a, w1b_, w1r = w1_tile(np.ascontiguousarray(w1[:, hs]))
        n1a, n1b_, n1r = w1_tile(np.ascontiguousarray(noise1[:, hs]))
        in_maps.append({
            "xtb": xtb,
            "w1a": w1a, "w1b": w1b_, "w1r": w1r,
            "n1a": n1a, "n1b": n1b_, "n1r": n1r,
            "s1h": np.ascontiguousarray((0.5 * s1[hs]).reshape(MT, 128).T),
            "b1m": np.ascontiguousarray(b1[hs].reshape(MT, 128).T),
            "w2g": w2_tile(np.ascontiguousarray(w2[hs, :])),
            "n2g": w2_tile(np.ascontiguousarray(noise2[hs, :])),
            "s2d": s2m,
            "b2d": b2m,
        })
    return in_maps


def kernel(x, w1, s1, b1, w2, s2, b2, noise1, noise2, _bench_out=None):
    """Full-input, full-output entry point. Shards across 8 NeuronCores."""
    nc = _get_nc()
    in_maps = _make_in_maps(x, w1, s1, b1, w2, s2, b2, noise1, noise2)
    res = run_bass_kernel_spmd(nc, in_maps, core_ids=list(range(N_CORES)))
    if _bench_out is not None:
        _bench_out.append(res)
    yT = np.empty((DOUT, B), dtype=np.float32)
    for c in range(N_CORES):
        out_c = np.asarray(res.results[c]["yo"]).astype(np.float32)
        for b in range(NB):
            cols = slice(b * 512, (b + 1) * 512)
            yT[c * 64:(c + 1) * 64, cols] = out_c[b, 0:64]
            yT[512 + c * 64:512 + (c + 1) * 64, cols] = out_c[b, 64:128]
    return np.ascontiguousarray(yT.T).astype(np.float32)


if __name__ == "__main__":
    nc = build_bass()
    print("built OK")


# revision 51
# speedup vs baseline: 1.0279x; 1.0219x over previous
"""Trainium2 Bass kernel for nn_Encoder (dense MLP with stochastic ternarization).

y = tanh(x @ (s1*T(w1,n1)) + b1) @ (s2*T(w2,n2)) + b2,  T(w,n) = (w-n>1) - (w-n<-1)

Sharding: tensor-parallel over the 16384 hidden dim across 8 cores. Each core
gets a 2048-wide hidden shard of w1/noise1/s1/b1 (column-sharded) and the
matching 2048-row shard of w2/noise2; x is replicated (host pre-transposed to
bf16, tiled per 512-batch block).

Kernel structure:
- Ternary weights in fp8e4 ({-2,0,+2} exact); PE takes fp8 stationary x bf16
  moving. 1 MiB contiguous weight DMA blocks (host pre-tiled); the first
  supply quarter is halved so the first matmul chains unblock early.
- Blocks 0/1 interleave layer-1 m-groups at supply-stripe granularity so PE
  consumption tracks the ternarize supply; blocks 2/3 run from resident
  weights.
- h stays in SBUF between layers. s2/b2 are applied at PSUM evacuation (b2 on
  core 0 only) and partials are stored bf16; each (block, half) goes through
  its own bf16 ReduceScatter(add) into DRAM scratch, DMA-copied to the output
  from the gpsimd queue (a sync-queue copy would head-of-line block y stores).
  Block 3 reduces in a single RS (per-op latency dominates at these sizes, so
  one op beats a serialized chain in the tail), and two 1 KiB pre-sync
  collectives during L1(b3) absorb accumulated inter-core skew beforehand.

Ternarization: q = w - noise (DVE), tanh(2^30*(q-1)) + tanh(2^30*(q+1)) (ACT)
== (q>1)-(q<-1) doubled; the factor 2 is folded into s1/s2 on the host.
"""

import sys

for _p in ("/opt/trn_rl_repo",):
    if _p not in sys.path:
        sys.path.insert(0, _p)

import numpy as np
import ml_dtypes

import concourse.bass as bass
import concourse.bacc as bacc
import concourse.mybir as mybir
import concourse.tile as tile
import concourse.bass_utils as _bass_utils
from concourse.bass_utils import run_bass_kernel_spmd

BF16 = mybir.dt.bfloat16
F32 = mybir.dt.float32
FP8 = mybir.dt.float8e4
NPBF16 = ml_dtypes.bfloat16

N_CORES = 8
B = 2048
DIN = 3072
DHID = 16384
DOUT = 1024
HSH = DHID // N_CORES   # 2048
DSH = DOUT // N_CORES   # 128

K1 = DIN // 128          # 24 contraction tiles, layer 1
KG1 = K1 // 4            # 6 groups of 4 k-tiles (1 MiB weight blocks)
K2 = HSH // 128          # 16 contraction tiles, layer 2
KG2 = K2 // 2            # 8 groups of 2 k2-tiles (1 MiB weight blocks)
NB = B // 512            # 4 batch blocks
MT = HSH // 128          # 16 hidden m-tiles
ND = DOUT // 128         # 8 dout tiles
QW = 512                 # ternarize quarter width
NQ = HSH // QW           # 4 quarters

BIGK = float(2 ** 30)

TANH = mybir.ActivationFunctionType.Tanh
MULT = mybir.AluOpType.mult
ADD = mybir.AluOpType.add


def build_bass():
    nc = bacc.Bacc("TRN2", target_bir_lowering=False, debug=False, num_devices=N_CORES)

    xtb = nc.dram_tensor("xtb", [NB, 128, K1, 512], BF16, kind="ExternalInput")
    # cols 0:256 as two contiguous 128-wide stripes, 256:512 as one 256-wide,
    # then full 512-wide quarters: contiguous DMA blocks in supply order
    w1a = nc.dram_tensor("w1a", [2, KG1, 128, 4, 128], F32, kind="ExternalInput")
    n1a = nc.dram_tensor("n1a", [2, KG1, 128, 4, 128], F32, kind="ExternalInput")
    w1b = nc.dram_tensor("w1b", [1, KG1, 128, 4, 256], F32, kind="ExternalInput")
    n1b = nc.dram_tensor("n1b", [1, KG1, 128, 4, 256], F32, kind="ExternalInput")
    w1r = nc.dram_tensor("w1r", [3, KG1, 128, 4, QW], F32, kind="ExternalInput")
    n1r = nc.dram_tensor("n1r", [3, KG1, 128, 4, QW], F32, kind="ExternalInput")
    s1h = nc.dram_tensor("s1h", [128, MT], F32, kind="ExternalInput")
    b1m = nc.dram_tensor("b1m", [128, MT], F32, kind="ExternalInput")
    w2g = nc.dram_tensor("w2g", [KG2, 128, 2, DOUT], F32, kind="ExternalInput")
    n2g = nc.dram_tensor("n2g", [KG2, 128, 2, DOUT], F32, kind="ExternalInput")
    s2d = nc.dram_tensor("s2d", [128, ND], F32, kind="ExternalInput")
    b2d = nc.dram_tensor("b2d", [128, ND], F32, kind="ExternalInput")

    # per-core output rows for block b:
    #   blocks 0-2: rows 0:64 = chunk of douts 0:512 (RS over d-tiles 0-3),
    #               rows 64:128 = chunk of douts 512:1024
    #   block 3: rows 0:128 = chunk of douts 0:1024 (single RS)
    yo = nc.dram_tensor("yo", [NB, 128, 512], BF16, kind="ExternalOutput")

    with tile.TileContext(nc) as tc:
        with (
            tc.tile_pool(name="const", bufs=1) as cpool,
            tc.tile_pool(name="dram", bufs=1, space="DRAM") as dpool,
            tc.tile_pool(name="t2w1", bufs=KG1) as t2pool,
            tc.tile_pool(name="t2w2", bufs=1) as t22pool,
            tc.tile_pool(name="stage", bufs=3) as spool,
            tc.tile_pool(name="act8", bufs=2) as apool,
            tc.tile_pool(name="xtn", bufs=2) as xpool,
            tc.tile_pool(name="hblk", bufs=2 * MT) as hpool,
            tc.tile_pool(name="yblk", bufs=4) as ypool,
            tc.tile_pool(name="ps1", bufs=6, space="PSUM") as pspool,
            tc.tile_pool(name="ps2", bufs=2, space="PSUM") as ps2pool,
        ):
            s1_sb = cpool.tile([128, MT], F32, tag="s1")
            b1_sb = cpool.tile([128, MT], F32, tag="b1")
            s2_sb = cpool.tile([128, ND], F32, tag="s2")
            b2_sb = cpool.tile([128, ND], F32, tag="b2")
            nc.scalar.dma_start(s1_sb[:], s1h[:, :])
            nc.scalar.dma_start(b1_sb[:], b1m[:, :])
            nc.scalar.dma_start(s2_sb[:], s2d[:, :])
            nc.scalar.dma_start(b2_sb[:], b2d[:, :])
            kneg = cpool.tile([128, 1], F32, tag="kneg")
            nc.vector.memset(kneg[:], -BIGK)
            kpos = cpool.tile([128, 1], F32, tag="kpos")
            nc.vector.memset(kpos[:], BIGK)

            # RS inputs/outputs: (block, segment). Two halves per block: the
            # first half's RS overlaps the second half's d-tile compute, so
            # the serial tail is one 512 KiB op instead of a 1 MiB op.
            def seg_rows(b):
                return [512, 512]

            yq = [[dpool.tile([r, 512], BF16, tag=f"yq{b}{i}",
                              name=f"yq_b{b}s{i}")
                   for i, r in enumerate(seg_rows(b))] for b in range(NB)]
            ro = [[dpool.tile([r // N_CORES, 512], BF16, tag=f"ro{b}{i}",
                              name=f"ro_b{b}s{i}")
                   for i, r in enumerate(seg_rows(b))] for b in range(NB)]
            dsrc = [dpool.tile([8, 64], BF16, tag=f"dsrc{i}",
                               name=f"dsrc{i}") for i in range(2)]
            ddst = [dpool.tile([1, 64], BF16, tag=f"ddst{i}",
                               name=f"ddst{i}") for i in range(2)]

            xtn_tiles = {}
            for b in (0, 1):
                xtn_tiles[b] = xpool.tile([128, K1, 512], BF16, tag="xtn",
                                          name=f"xtn{b}")
                nc.scalar.dma_start(xtn_tiles[b][:], xtb[b])

            # ---- ternarize (1 MiB blocks) ----
            t2g = [t2pool.tile([128, 4, HSH], FP8, tag="t2", name=f"t2g_{kg}")
                   for kg in range(KG1)]
            t22 = t22pool.tile([128, K2, DOUT], FP8, tag="t22")

            def tern_block(dst_ap, w_src, n_src, sub_k, fd):
                w_t = spool.tile([128, sub_k, fd], F32, tag="w")
                nc.sync.dma_start(w_t[:], w_src)
                n_t = spool.tile([128, sub_k, fd], F32, tag="n")
                nc.sync.dma_start(n_t[:], n_src)
                nc.vector.tensor_sub(w_t[:], w_t[:], n_t[:])
                a1 = apool.tile([128, sub_k, fd], FP8, tag="a1")
                nc.scalar.activation(a1[:], w_t[:], TANH, bias=kneg[:, 0:1], scale=BIGK)
                a2 = apool.tile([128, sub_k, fd], FP8, tag="a2")
                nc.scalar.activation(a2[:], w_t[:], TANH, bias=kpos[:, 0:1], scale=BIGK)
                nc.vector.tensor_add(dst_ap, a1[:], a2[:])

            def presync(i, dep_tile):
                # tiny store creates the trigger-time dependency; the 1 KiB
                # ReduceScatter syncs the 8 cores' CC streams under compute
                nc.sync.dma_start(dsrc[i][:], dep_tile[0:8, 0:64])
                nc.gpsimd.collective_compute(
                    "ReduceScatter",
                    mybir.AluOpType.add,
                    replica_groups=[list(range(N_CORES))],
                    ins=[dsrc[i].opt()],
                    outs=[ddst[i].opt()],
                )

            # supply stripes: narrow contiguous blocks first (fast start),
            # then full 512-wide quarters
            SUPPLY = [("a", 0, 0, 128), ("a", 1, 128, 128), ("b", 0, 256, 256),
                      ("r", 0, 512, 512), ("r", 1, 1024, 512), ("r", 2, 1536, 512)]
            srcs = {"a": (w1a, n1a), "b": (w1b, n1b), "r": (w1r, n1r)}
            for t, i, c0, cw in SUPPLY:
                ws, ns = srcs[t]
                for kg in range(KG1):
                    tern_block(
                        t2g[kg][:, :, c0:c0 + cw],
                        ws[i, kg], ns[i, kg], 4, cw,
                    )
            for kg in range(KG2):
                tern_block(
                    t22[:, kg * 2:(kg + 1) * 2, :],
                    w2g[kg], n2g[kg], 2, DOUT,
                )

            # ---- compute ----
            h_sets = {0: [], 1: [], 2: [], 3: []}

            MGROUPS = [(0, 1), (1, 1), (2, 2), (4, 4), (8, 4), (12, 4)]

            def layer1_mgroup(b, g):
                xtn = xtn_tiles[b]
                m0, mw = MGROUPS[g]
                for m in range(m0, m0 + mw):
                    ps = pspool.tile([128, 512], F32, tag="ps")
                    for k in range(K1):
                        nc.tensor.matmul(
                            ps[:],
                            t2g[k // 4][:, k % 4, m * 128:(m + 1) * 128],
                            xtn[:, k, :],
                            start=(k == 0), stop=(k == K1 - 1))
                    h_m = hpool.tile([128, 512], BF16, tag="h")
                    nc.scalar.activation(
                        h_m[:], ps[:], TANH,
                        bias=b1_sb[:, m:m + 1], scale=s1_sb[:, m:m + 1],
                    )
                    h_sets[b].append(h_m)

            def layer2_block(b):
                seg_of_d = [0, 0, 0, 0, 1, 1, 1, 1]
                part_of_d = [0, 1, 2, 3, 0, 1, 2, 3]
                last_of_seg = [3, 7]
                for d in range(ND):
                    p = ps2pool.tile([128, 512], F32, tag="ps2")
                    for k2 in range(K2):
                        nc.tensor.matmul(p[:], t22[:, k2, d * 128:(d + 1) * 128],
                                         h_sets[b][k2][:],
                                         start=(k2 == 0), stop=(k2 == K2 - 1))
                    y_sb = ypool.tile([128, 512], BF16, tag="y")
                    nc.vector.tensor_scalar(
                        y_sb[:], p[:], s2_sb[:, d:d + 1], b2_sb[:, d:d + 1],
                        MULT, ADD,
                    )
                    seg, part = seg_of_d[d], part_of_d[d]
                    nc.scalar.dma_start(
                        yq[b][seg][part * 128:(part + 1) * 128, :], y_sb[:],
                    )
                    if d in last_of_seg:
                        nc.gpsimd.collective_compute(
                            "ReduceScatter",
                            mybir.AluOpType.add,
                            replica_groups=[list(range(N_CORES))],
                            ins=[yq[b][seg].opt()],
                            outs=[ro[b][seg].opt()],
                        )
                # yo copies last on the gpsimd queue (they wait on RS
                # completion; nothing time-critical queues behind them there)
                r0 = 0
                for i, r in enumerate(seg_rows(b)):
                    rc = r // N_CORES
                    nc.gpsimd.dma_start(yo[b, r0:r0 + rc, :], ro[b][i][:])
                    r0 += rc

            # blocks 0/1 with supply-interleaved layer-1
            for g in range(len(MGROUPS)):
                layer1_mgroup(0, g)
                layer1_mgroup(1, g)
            layer2_block(0)
            layer2_block(1)

            # blocks 2/3 from resident weights
            for b in (2, 3):
                xtn_tiles[b] = xpool.tile([128, K1, 512], BF16, tag="xtn",
                                          name=f"xtn{b}")
                nc.scalar.dma_start(xtn_tiles[b][:], xtb[b])
                for g in range(len(MGROUPS)):
                    layer1_mgroup(b, g)
                    # presync early enough that its CC op completes before
                    # the block's first RS data is ready (an in-flight CC op
                    # delays the next trigger — seen when presync ran at g4)
                    if b == 3 and g in (0, 3):
                        presync(0 if g == 0 else 1, h_sets[3][-1])
                layer2_block(b)
            # g==4 ends at m11; the second presync completes under the m12-15
            # and L2(b3) compute, so the final RS pays only residual skew

    nc.compile()
    return nc


_NC_CACHE = {}


def _get_nc():
    if "nc" not in _NC_CACHE:
        _NC_CACHE["nc"] = build_bass()
    return _NC_CACHE["nc"]


def _make_in_maps(x, w1, s1, b1, w2, s2, b2, noise1, noise2):
    x = np.asarray(x, dtype=np.float32)
    w1 = np.asarray(w1, dtype=np.float32)
    s1 = np.asarray(s1, dtype=np.float32)
    b1 = np.asarray(b1, dtype=np.float32)
    w2 = np.asarray(w2, dtype=np.float32)
    s2 = np.asarray(s2, dtype=np.float32)
    b2 = np.asarray(b2, dtype=np.float32)
    noise1 = np.asarray(noise1, dtype=np.float32)
    noise2 = np.asarray(noise2, dtype=np.float32)

    xT = x.T.astype(NPBF16)
    xtb = np.ascontiguousarray(xT.reshape(K1, 128, NB, 512).transpose(2, 1, 0, 3))

    def w1_tile(w):   # [din, HSH] -> (a [2,KG1,128,4,128], b [1,...,256], r [3,...,512])
        wk = w.reshape(KG1, 4, 128, HSH)
        a = np.empty((2, KG1, 128, 4, 128), dtype=np.float32)
        bt = np.empty((1, KG1, 128, 4, 256), dtype=np.float32)
        r = np.empty((3, KG1, 128, 4, QW), dtype=np.float32)
        for i in range(2):
            a[i] = wk[:, :, :, i * 128:(i + 1) * 128].transpose(0, 2, 1, 3)
        bt[0] = wk[:, :, :, 256:512].transpose(0, 2, 1, 3)
        for i in range(3):
            r[i] = wk[:, :, :, 512 + i * 512:512 + (i + 1) * 512].transpose(0, 2, 1, 3)
        return (np.ascontiguousarray(a), np.ascontiguousarray(bt),
                np.ascontiguousarray(r))

    def w2_tile(w):   # [HSH, DOUT] -> [KG2, 128, 2, DOUT]
        return np.ascontiguousarray(
            w.reshape(KG2, 2, 128, DOUT).transpose(0, 2, 1, 3))

    in_maps = []
    for c in range(N_CORES):
        hs = slice(c * HSH, (c + 1) * HSH)
        s2m = np.ascontiguousarray((0.5 * s2).reshape(ND, 128).T)
        b2m = np.ascontiguousarray(b2.reshape(ND, 128).T) if c == 0 else \
            np.zeros((128, ND), dtype=np.float32)
        w1a, w1b_, w1r = w1_tile(np.ascontiguousarray(w1[:, hs]))
        n1a, n1b_, n1r = w1_tile(np.ascontiguousarray(noise1[:, hs]))
        in_maps.append({
            "xtb": xtb,
            "w1a": w1a, "w1b": w1b_, "w1r": w1r,
            "n1a": n1a, "n1b": n1b_, "n1r": n1r,
            "s1h": np.ascontiguousarray((0.5 * s1[hs]).reshape(MT, 128).T),
            "b1m": np.ascontiguousarray(b1[hs].reshape(MT, 128).T),
            "w2g": w2_tile(np.ascontiguousarray(w2[hs, :])),
            "n2g": w2_tile(np.ascontiguousarray(noise2[hs, :])),
            "s2d": s2m,
            "b2d": b2m,
        })
    return in_maps


def kernel(x, w1, s1, b1, w2, s2, b2, noise1, noise2, _bench_out=None):
    """Full-input, full-output entry point. Shards across 8 NeuronCores."""
    nc = _get_nc()
    in_maps = _make_in_maps(x, w1, s1, b1, w2, s2, b2, noise1, noise2)
    res = run_bass_kernel_spmd(nc, in_maps, core_ids=list(range(N_CORES)))
    if _bench_out is not None:
        _bench_out.append(res)
    yT = np.empty((DOUT, B), dtype=np.float32)
    for c in range(N_CORES):
        out_c = np.asarray(res.results[c]["yo"]).astype(np.float32)
        for b in range(NB):
            cols = slice(b * 512, (b + 1) * 512)
            yT[c * 64:(c + 1) * 64, cols] = out_c[b, 0:64]
            yT[512 + c * 64:512 + (c + 1) * 64, cols] = out_c[b, 64:128]
    return np.ascontiguousarray(yT.T).astype(np.float32)


if __name__ == "__main__":
    nc = build_bass()
    print("built OK")


# revision 55
# speedup vs baseline: 1.0360x; 1.0079x over previous
"""Trainium2 Bass kernel for nn_Encoder (dense MLP with stochastic ternarization).

y = tanh(x @ (s1*T(w1,n1)) + b1) @ (s2*T(w2,n2)) + b2,  T(w,n) = (w-n>1) - (w-n<-1)

Sharding: tensor-parallel over the 16384 hidden dim across 8 cores. Each core
gets a 2048-wide hidden shard of w1/noise1/s1/b1 (column-sharded) and the
matching 2048-row shard of w2/noise2; x is replicated (host pre-transposed to
bf16, tiled per 512-batch block).

Kernel structure:
- Ternary weights in fp8e4 ({-2,0,+2} exact); PE takes fp8 stationary x bf16
  moving. 1 MiB contiguous weight DMA blocks (host pre-tiled); the first
  supply quarter is halved so the first matmul chains unblock early.
- Blocks 0/1 interleave layer-1 m-groups at supply-stripe granularity so PE
  consumption tracks the ternarize supply; blocks 2/3 run from resident
  weights.
- h stays in SBUF between layers. s2/b2 are applied at PSUM evacuation (b2 on
  core 0 only) and partials are stored bf16; each (block, half) goes through
  its own bf16 ReduceScatter(add) into DRAM scratch, DMA-copied to the output
  from the gpsimd queue (a sync-queue copy would head-of-line block y stores).
  Block 3 reduces in a single RS (per-op latency dominates at these sizes, so
  one op beats a serialized chain in the tail), and two 1 KiB pre-sync
  collectives during L1(b3) absorb accumulated inter-core skew beforehand.

Ternarization: q = w - noise (DVE), tanh(2^30*(q-1)) + tanh(2^30*(q+1)) (ACT)
== (q>1)-(q<-1) doubled; the factor 2 is folded into s1/s2 on the host.
"""

import sys

for _p in ("/opt/trn_rl_repo",):
    if _p not in sys.path:
        sys.path.insert(0, _p)

import numpy as np
import ml_dtypes

import concourse.bass as bass
import concourse.bacc as bacc
import concourse.mybir as mybir
import concourse.tile as tile
import concourse.bass_utils as _bass_utils
from concourse.bass_utils import run_bass_kernel_spmd

BF16 = mybir.dt.bfloat16
F32 = mybir.dt.float32
FP8 = mybir.dt.float8e4
NPBF16 = ml_dtypes.bfloat16

N_CORES = 8
B = 2048
DIN = 3072
DHID = 16384
DOUT = 1024
HSH = DHID // N_CORES   # 2048
DSH = DOUT // N_CORES   # 128

K1 = DIN // 128          # 24 contraction tiles, layer 1
KG1 = K1 // 4            # 6 groups of 4 k-tiles (1 MiB weight blocks)
K2 = HSH // 128          # 16 contraction tiles, layer 2
KG2 = K2 // 2            # 8 groups of 2 k2-tiles (1 MiB weight blocks)
NB = B // 512            # 4 batch blocks
MT = HSH // 128          # 16 hidden m-tiles
ND = DOUT // 128         # 8 dout tiles
QW = 512                 # ternarize quarter width
NQ = HSH // QW           # 4 quarters

BIGK = float(2 ** 30)

TANH = mybir.ActivationFunctionType.Tanh
MULT = mybir.AluOpType.mult
ADD = mybir.AluOpType.add


def build_bass():
    nc = bacc.Bacc("TRN2", target_bir_lowering=False, debug=False, num_devices=N_CORES)

    xtb = nc.dram_tensor("xtb", [NB, 128, K1, 512], BF16, kind="ExternalInput")
    # cols 0:256 as two contiguous 128-wide stripes, 256:512 as one 256-wide,
    # then full 512-wide quarters: contiguous DMA blocks in supply order
    w1a = nc.dram_tensor("w1a", [2, KG1, 128, 4, 128], F32, kind="ExternalInput")
    n1a = nc.dram_tensor("n1a", [2, KG1, 128, 4, 128], F32, kind="ExternalInput")
    w1b = nc.dram_tensor("w1b", [1, KG1, 128, 4, 256], F32, kind="ExternalInput")
    n1b = nc.dram_tensor("n1b", [1, KG1, 128, 4, 256], F32, kind="ExternalInput")
    w1r = nc.dram_tensor("w1r", [3, KG1, 128, 4, QW], F32, kind="ExternalInput")
    n1r = nc.dram_tensor("n1r", [3, KG1, 128, 4, QW], F32, kind="ExternalInput")
    s1h = nc.dram_tensor("s1h", [128, MT], F32, kind="ExternalInput")
    b1m = nc.dram_tensor("b1m", [128, MT], F32, kind="ExternalInput")
    w2g = nc.dram_tensor("w2g", [KG2, 128, 2, DOUT], F32, kind="ExternalInput")
    n2g = nc.dram_tensor("n2g", [KG2, 128, 2, DOUT], F32, kind="ExternalInput")
    s2d = nc.dram_tensor("s2d", [128, ND], F32, kind="ExternalInput")
    b2d = nc.dram_tensor("b2d", [128, ND], F32, kind="ExternalInput")

    # per-core output rows for block b:
    #   blocks 0-2: rows 0:64 = chunk of douts 0:512 (RS over d-tiles 0-3),
    #               rows 64:128 = chunk of douts 512:1024
    #   block 3: rows 0:128 = chunk of douts 0:1024 (single RS)
    yo = nc.dram_tensor("yo", [NB, 128, 512], BF16, kind="ExternalOutput")

    with tile.TileContext(nc) as tc:
        with (
            tc.tile_pool(name="const", bufs=1) as cpool,
            tc.tile_pool(name="dram", bufs=1, space="DRAM") as dpool,
            tc.tile_pool(name="t2w1", bufs=KG1) as t2pool,
            tc.tile_pool(name="t2w2", bufs=1) as t22pool,
            tc.tile_pool(name="stagew", bufs=4) as wpool,
            tc.tile_pool(name="stagen", bufs=2) as npool,
            tc.tile_pool(name="act8", bufs=2) as apool,
            tc.tile_pool(name="xtn", bufs=2) as xpool,
            tc.tile_pool(name="hblk", bufs=2 * MT) as hpool,
            tc.tile_pool(name="yblk", bufs=2) as ypool,
            tc.tile_pool(name="ps1", bufs=6, space="PSUM") as pspool,
            tc.tile_pool(name="ps2", bufs=2, space="PSUM") as ps2pool,
        ):
            s1_sb = cpool.tile([128, MT], F32, tag="s1")
            b1_sb = cpool.tile([128, MT], F32, tag="b1")
            s2_sb = cpool.tile([128, ND], F32, tag="s2")
            b2_sb = cpool.tile([128, ND], F32, tag="b2")
            nc.scalar.dma_start(s1_sb[:], s1h[:, :])
            nc.scalar.dma_start(b1_sb[:], b1m[:, :])
            nc.scalar.dma_start(s2_sb[:], s2d[:, :])
            nc.scalar.dma_start(b2_sb[:], b2d[:, :])
            kneg = cpool.tile([128, 1], F32, tag="kneg")
            nc.vector.memset(kneg[:], -BIGK)
            kpos = cpool.tile([128, 1], F32, tag="kpos")
            nc.vector.memset(kpos[:], BIGK)

            # RS inputs/outputs: (block, segment). Two halves per block: the
            # first half's RS overlaps the second half's d-tile compute, so
            # the serial tail is one 512 KiB op instead of a 1 MiB op.
            def seg_rows(b):
                return [512, 512]

            yq = [[dpool.tile([r, 512], BF16, tag=f"yq{b}{i}",
                              name=f"yq_b{b}s{i}")
                   for i, r in enumerate(seg_rows(b))] for b in range(NB)]
            ro = [[dpool.tile([r // N_CORES, 512], BF16, tag=f"ro{b}{i}",
                              name=f"ro_b{b}s{i}")
                   for i, r in enumerate(seg_rows(b))] for b in range(NB)]
            dsrc = [dpool.tile([8, 64], BF16, tag=f"dsrc{i}",
                               name=f"dsrc{i}") for i in range(2)]
            ddst = [dpool.tile([1, 64], BF16, tag=f"ddst{i}",
                               name=f"ddst{i}") for i in range(2)]

            xtn_tiles = {}
            for b in (0, 1):
                xtn_tiles[b] = xpool.tile([128, K1, 512], BF16, tag="xtn",
                                          name=f"xtn{b}")
                nc.scalar.dma_start(xtn_tiles[b][:], xtb[b])

            # ---- ternarize (1 MiB blocks) ----
            t2g = [t2pool.tile([128, 4, HSH], FP8, tag="t2", name=f"t2g_{kg}")
                   for kg in range(KG1)]
            t22 = t22pool.tile([128, K2, DOUT], FP8, tag="t22")

            def tern_block(dst_ap, w_src, n_src, sub_k, fd):
                w_t = wpool.tile([128, sub_k, fd], F32, tag="w")
                nc.sync.dma_start(w_t[:], w_src)
                n_t = npool.tile([128, sub_k, fd], F32, tag="n")
                nc.sync.dma_start(n_t[:], n_src)
                nc.vector.tensor_sub(w_t[:], w_t[:], n_t[:])
                a1 = apool.tile([128, sub_k, fd], FP8, tag="a1")
                nc.scalar.activation(a1[:], w_t[:], TANH, bias=kneg[:, 0:1], scale=BIGK)
                a2 = apool.tile([128, sub_k, fd], FP8, tag="a2")
                nc.scalar.activation(a2[:], w_t[:], TANH, bias=kpos[:, 0:1], scale=BIGK)
                nc.vector.tensor_add(dst_ap, a1[:], a2[:])

            def presync(i, dep_tile):
                # tiny store creates the trigger-time dependency; the 1 KiB
                # ReduceScatter syncs the 8 cores' CC streams under compute
                nc.sync.dma_start(dsrc[i][:], dep_tile[0:8, 0:64])
                nc.gpsimd.collective_compute(
                    "ReduceScatter",
                    mybir.AluOpType.add,
                    replica_groups=[list(range(N_CORES))],
                    ins=[dsrc[i].opt()],
                    outs=[ddst[i].opt()],
                )

            # supply stripes: narrow contiguous blocks first (fast start),
            # then full 512-wide quarters
            SUPPLY = [("a", 0, 0, 128), ("a", 1, 128, 128), ("b", 0, 256, 256),
                      ("r", 0, 512, 512), ("r", 1, 1024, 512), ("r", 2, 1536, 512)]
            srcs = {"a": (w1a, n1a), "b": (w1b, n1b), "r": (w1r, n1r)}
            for t, i, c0, cw in SUPPLY:
                ws, ns = srcs[t]
                for kg in range(KG1):
                    tern_block(
                        t2g[kg][:, :, c0:c0 + cw],
                        ws[i, kg], ns[i, kg], 4, cw,
                    )
            for kg in range(KG2):
                tern_block(
                    t22[:, kg * 2:(kg + 1) * 2, :],
                    w2g[kg], n2g[kg], 2, DOUT,
                )

            # ---- compute ----
            h_sets = {0: [], 1: [], 2: [], 3: []}

            MGROUPS = [(0, 1), (1, 1), (2, 2), (4, 4), (8, 4), (12, 4)]

            def layer1_mgroup(b, g):
                xtn = xtn_tiles[b]
                m0, mw = MGROUPS[g]
                for m in range(m0, m0 + mw):
                    ps = pspool.tile([128, 512], F32, tag="ps")
                    for k in range(K1):
                        nc.tensor.matmul(
                            ps[:],
                            t2g[k // 4][:, k % 4, m * 128:(m + 1) * 128],
                            xtn[:, k, :],
                            start=(k == 0), stop=(k == K1 - 1))
                    h_m = hpool.tile([128, 512], BF16, tag="h")
                    nc.scalar.activation(
                        h_m[:], ps[:], TANH,
                        bias=b1_sb[:, m:m + 1], scale=s1_sb[:, m:m + 1],
                    )
                    h_sets[b].append(h_m)

            def layer2_block(b):
                seg_of_d = [0, 0, 0, 0, 1, 1, 1, 1]
                part_of_d = [0, 1, 2, 3, 0, 1, 2, 3]
                last_of_seg = [3, 7]
                for d in range(ND):
                    p = ps2pool.tile([128, 512], F32, tag="ps2")
                    for k2 in range(K2):
                        nc.tensor.matmul(p[:], t22[:, k2, d * 128:(d + 1) * 128],
                                         h_sets[b][k2][:],
                                         start=(k2 == 0), stop=(k2 == K2 - 1))
                    y_sb = ypool.tile([128, 512], BF16, tag="y")
                    nc.vector.tensor_scalar(
                        y_sb[:], p[:], s2_sb[:, d:d + 1], b2_sb[:, d:d + 1],
                        MULT, ADD,
                    )
                    seg, part = seg_of_d[d], part_of_d[d]
                    nc.scalar.dma_start(
                        yq[b][seg][part * 128:(part + 1) * 128, :], y_sb[:],
                    )
                    if d in last_of_seg:
                        nc.gpsimd.collective_compute(
                            "ReduceScatter",
                            mybir.AluOpType.add,
                            replica_groups=[list(range(N_CORES))],
                            ins=[yq[b][seg].opt()],
                            outs=[ro[b][seg].opt()],
                        )
                # yo copies last on the gpsimd queue (they wait on RS
                # completion; nothing time-critical queues behind them there)
                r0 = 0
                for i, r in enumerate(seg_rows(b)):
                    rc = r // N_CORES
                    nc.gpsimd.dma_start(yo[b, r0:r0 + rc, :], ro[b][i][:])
                    r0 += rc

            # blocks 0/1 with supply-interleaved layer-1
            for g in range(len(MGROUPS)):
                layer1_mgroup(0, g)
                layer1_mgroup(1, g)
            layer2_block(0)
            layer2_block(1)

            # blocks 2/3 from resident weights
            for b in (2, 3):
                xtn_tiles[b] = xpool.tile([128, K1, 512], BF16, tag="xtn",
                                          name=f"xtn{b}")
                nc.scalar.dma_start(xtn_tiles[b][:], xtb[b])
                for g in range(len(MGROUPS)):
                    layer1_mgroup(b, g)
                    # presync early enough that its CC op completes before
                    # the block's first RS data is ready (an in-flight CC op
                    # delays the next trigger — seen when presync ran at g4)
                    if b == 3 and g in (0, 3):
                        presync(0 if g == 0 else 1, h_sets[3][-1])
                layer2_block(b)
            # g==4 ends at m11; the second presync completes under the m12-15
            # and L2(b3) compute, so the final RS pays only residual skew

    nc.compile()
    return nc


_NC_CACHE = {}


def _get_nc():
    if "nc" not in _NC_CACHE:
        _NC_CACHE["nc"] = build_bass()
    return _NC_CACHE["nc"]


def _make_in_maps(x, w1, s1, b1, w2, s2, b2, noise1, noise2):
    x = np.asarray(x, dtype=np.float32)
    w1 = np.asarray(w1, dtype=np.float32)
    s1 = np.asarray(s1, dtype=np.float32)
    b1 = np.asarray(b1, dtype=np.float32)
    w2 = np.asarray(w2, dtype=np.float32)
    s2 = np.asarray(s2, dtype=np.float32)
    b2 = np.asarray(b2, dtype=np.float32)
    noise1 = np.asarray(noise1, dtype=np.float32)
    noise2 = np.asarray(noise2, dtype=np.float32)

    xT = x.T.astype(NPBF16)
    xtb = np.ascontiguousarray(xT.reshape(K1, 128, NB, 512).transpose(2, 1, 0, 3))

    def w1_tile(w):   # [din, HSH] -> (a [2,KG1,128,4,128], b [1,...,256], r [3,...,512])
        wk = w.reshape(KG1, 4, 128, HSH)
        a = np.empty((2, KG1, 128, 4, 128), dtype=np.float32)
        bt = np.empty((1, KG1, 128, 4, 256), dtype=np.float32)
        r = np.empty((3, KG1, 128, 4, QW), dtype=np.float32)
        for i in range(2):
            a[i] = wk[:, :, :, i * 128:(i + 1) * 128].transpose(0, 2, 1, 3)
        bt[0] = wk[:, :, :, 256:512].transpose(0, 2, 1, 3)
        for i in range(3):
            r[i] = wk[:, :, :, 512 + i * 512:512 + (i + 1) * 512].transpose(0, 2, 1, 3)
        return (np.ascontiguousarray(a), np.ascontiguousarray(bt),
                np.ascontiguousarray(r))

    def w2_tile(w):   # [HSH, DOUT] -> [KG2, 128, 2, DOUT]
        return np.ascontiguousarray(
            w.reshape(KG2, 2, 128, DOUT).transpose(0, 2, 1, 3))

    in_maps = []
    for c in range(N_CORES):
        hs = slice(c * HSH, (c + 1) * HSH)
        s2m = np.ascontiguousarray((0.5 * s2).reshape(ND, 128).T)
        b2m = np.ascontiguousarray(b2.reshape(ND, 128).T) if c == 0 else \
            np.zeros((128, ND), dtype=np.float32)
        w1a, w1b_, w1r = w1_tile(np.ascontiguousarray(w1[:, hs]))
        n1a, n1b_, n1r = w1_tile(np.ascontiguousarray(noise1[:, hs]))
        in_maps.append({
            "xtb": xtb,
            "w1a": w1a, "w1b": w1b_, "w1r": w1r,
            "n1a": n1a, "n1b": n1b_, "n1r": n1r,
            "s1h": np.ascontiguousarray((0.5 * s1[hs]).reshape(MT, 128).T),
            "b1m": np.ascontiguousarray(b1[hs].reshape(MT, 128).T),
            "w2g": w2_tile(np.ascontiguousarray(w2[hs, :])),
            "n2g": w2_tile(np.ascontiguousarray(noise2[hs, :])),
            "s2d": s2m,
            "b2d": b2m,
        })
    return in_maps


def kernel(x, w1, s1, b1, w2, s2, b2, noise1, noise2, _bench_out=None):
    """Full-input, full-output entry point. Shards across 8 NeuronCores."""
    nc = _get_nc()
    in_maps = _make_in_maps(x, w1, s1, b1, w2, s2, b2, noise1, noise2)
    res = run_bass_kernel_spmd(nc, in_maps, core_ids=list(range(N_CORES)))
    if _bench_out is not None:
        _bench_out.append(res)
    yT = np.empty((DOUT, B), dtype=np.float32)
    for c in range(N_CORES):
        out_c = np.asarray(res.results[c]["yo"]).astype(np.float32)
        for b in range(NB):
            cols = slice(b * 512, (b + 1) * 512)
            yT[c * 64:(c + 1) * 64, cols] = out_c[b, 0:64]
            yT[512 + c * 64:512 + (c + 1) * 64, cols] = out_c[b, 64:128]
    return np.ascontiguousarray(yT.T).astype(np.float32)


if __name__ == "__main__":
    nc = build_bass()
    print("built OK")
